# revision 35
# baseline (speedup 1.0000x reference)
"""Multi-head self-attention (B=2, S=2048, D=1024, H=16) on 8 TRN2 NeuronCores.

Sharding: head-parallel — 2 heads per core. Each core computes Q/K/V
projections for its 2 heads over all B*S tokens, full (non-causal)
softmax attention for its 4 (batch, head) units, and a partial output
projection y_c = sum_h out_h @ wo[h]. Host sums the 8 partial outputs.
The host also pre-transposes x to xT (pure layout prep) so the device
reads the contraction dim on partitions directly.

Device dataflow (head-dim on partitions):
  q2t/k2t/v2t [128=2*64, T] = w[:,2heads]^T @ xT      (PSUM accum over D)
  v2t --PE transpose--> vnat [k, d] (+ ones column -> denominator row)
  scoresT[k, q] = K Q^T  (contract d=64), exp on ACT (scale=1/8 folded in)
  poT[d+1, q] += vnat^T @ exp  accumulated over k tiles  (PSUM)
  out2t[:, q] = poT[0:64] * (1/poT[64]) broadcast (K=1 matmul + DVE mul)
  y[s, n] = out2t[:, s-tile]^T @ wo2   (contract 128 = 2 heads * 64)

Matmuls run as float32r (tfloat32 datapath, 1 cycle/row); every operand
tile is produced with an fp32r-rounding instruction as the BIR verifier
requires. PSUM accumulation stays fp32.

Emission is phase-interleaved so the PE always has independent filler
work while the ACT engine grinds through the exps: projections for
batch 1 ride along with batch 0's attention, and batch 0's output
projection rides along with batch 1's attention.
"""

import numpy as np
from contextlib import ExitStack

import concourse.bass as bass
import concourse.tile as tile
from concourse import bacc, mybir
from concourse.bass_utils import run_bass_kernel_spmd
from concourse.masks import make_identity

F32 = mybir.dt.float32
F32R = mybir.dt.float32r
AF = mybir.ActivationFunctionType

N_CORES = 8
D_MODEL = 1024
NUM_HEADS = 16
DEPTH = 64
HEADS_PER_CORE = NUM_HEADS // N_CORES  # 2
B_FULL = 2
S_FULL = 2048


def build_program(T=4096, D=1024, S=2048, dh=64, hc=2, with_qkv_bias=False,
                  with_o_bias=False, use_f32r=True, dma_f32r=True):
    """Build the SPMD Bass program for one core (hc heads).

    T: total tokens (B*S); D: model dim; S: seq len per batch; dh: head depth;
    hc: heads per core. Requires hc*dh == 128, D % 128 == 0, S % 512 == 0,
    T % S == 0. dma_f32r: DMA x directly into fp32r tiles (if the verifier
    allows DMA producers); else DMA to fp32 and round via DVE copy.
    """
    d2 = hc * dh
    assert d2 == 128 and D % 128 == 0 and S % 512 == 0 and T % S == 0
    nb = T // S            # batches
    ndc = D // 128         # D chunks (contraction tiles)
    cpb = S // 512         # 512-token chunks per batch
    KT = S // 128          # k tiles per (b,h) unit
    QC = S // 512          # 512-wide q chunks per batch
    NJ = min(512, D)
    scale = 1.0 / float(np.sqrt(dh))
    MDT = F32R if use_f32r else F32

    nc = bacc.Bacc("TRN2", target_bir_lowering=False, debug=False,
                   num_devices=N_CORES)

    xt_d = nc.dram_tensor("xt", [D, T], F32R if dma_f32r else F32,
                          kind="ExternalInput").ap()
    wq_d = nc.dram_tensor("wq", [D, d2], F32, kind="ExternalInput").ap()
    wk_d = nc.dram_tensor("wk", [D, d2], F32, kind="ExternalInput").ap()
    wv_d = nc.dram_tensor("wv", [D, d2], F32, kind="ExternalInput").ap()
    wo_d = nc.dram_tensor("wo", [d2, D], F32, kind="ExternalInput").ap()
    if with_qkv_bias:
        bq_d = nc.dram_tensor("bq", [d2, 1], F32, kind="ExternalInput").ap()
        bk_d = nc.dram_tensor("bk", [d2, 1], F32, kind="ExternalInput").ap()
        bv_d = nc.dram_tensor("bv", [d2, 1], F32, kind="ExternalInput").ap()
    if with_o_bias:
        bo_d = nc.dram_tensor("bo", [1, D], F32, kind="ExternalInput").ap()
    y_d = nc.dram_tensor("y", [T, D], F32, kind="ExternalOutput").ap()

    xt_view = xt_d.rearrange("(dc p) t -> p dc t", p=128)

    with tile.TileContext(nc) as tc, ExitStack() as ctx:
        singles = ctx.enter_context(tc.tile_pool(name="singles", bufs=1))
        xtpool = ctx.enter_context(tc.tile_pool(name="xtpool", bufs=3))
        v2pool = ctx.enter_context(tc.tile_pool(name="v2pool", bufs=2))
        epool = ctx.enter_context(tc.tile_pool(name="epool", bufs=4))
        ysb = ctx.enter_context(tc.tile_pool(name="ysb", bufs=3))
        rcpool = ctx.enter_context(tc.tile_pool(name="rcpool", bufs=2))
        # PSUM budget (8 banks): sc 4x[128,512]=4, ps 2x[128,512]=2,
        # po 2x[65,512]=2
        pspool = ctx.enter_context(tc.tile_pool(name="ps", bufs=2, space="PSUM"))
        psO = ctx.enter_context(tc.tile_pool(name="psO", bufs=2, space="PSUM"))
        posb = ctx.enter_context(tc.tile_pool(name="posb", bufs=2))

        ident = singles.tile([128, 128], F32)
        make_identity(nc, ident[:])
        ones1f = singles.tile([1, dh], F32)
        nc.vector.memset(ones1f[:], 1.0)
        ones1 = singles.tile([1, dh], MDT)
        nc.vector.tensor_copy(ones1[:], ones1f[:])

        # weights: load fp32, round on-chip to the matmul dtype
        w_sb = []
        with tc.tile_pool(name="wraw", bufs=2) as wraw:
            for name, wd in (("wqs", wq_d), ("wks", wk_d), ("wvs", wv_d)):
                raw = wraw.tile([128, ndc, d2], F32, tag="wr", name=f"raw_{name}")
                nc.sync.dma_start(out=raw[:],
                                  in_=wd.rearrange("(dc p) m -> p dc m", p=128))
                t = singles.tile([128, ndc, d2], MDT, tag=name, name=name)
                nc.vector.tensor_copy(t[:], raw[:])
                w_sb.append(t)
            raw = wraw.tile([d2, D], F32, tag="wr", name="raw_wo")
            nc.sync.dma_start(out=raw[:], in_=wo_d)
            wo_sb = singles.tile([d2, D], MDT)
            nc.vector.tensor_copy(wo_sb[:], raw[:])

        b_sb = [None, None, None]
        if with_qkv_bias:
            for i, bd in enumerate((bq_d, bk_d, bv_d)):
                t = singles.tile([d2, 1], F32, tag=f"b{i}", name=f"b{i}")
                nc.sync.dma_start(out=t[:], in_=bd)
                b_sb[i] = t
        bo_sb = None
        if with_o_bias:
            bo_sb = singles.tile([128, D], F32)
            nc.gpsimd.dma_start(out=bo_sb[:], in_=bo_d.partition_broadcast(128))

        # Q stored zero-padded per head: q2tz[h] has head h's Q^T on its own
        # 64 partitions and ZEROS on the other 64. The score matmul then runs
        # with the full [128,128] two-head K tile as stationary (K=128
        # contraction) — fp32r at K=64 is half-rate, K=128 is full-rate —
        # and the zero rows cancel the other head's contribution.
        q2tz = [singles.tile([128, T], MDT, tag=f"q2tz{h}", name=f"q2tz{h}")
                for h in range(hc)]
        if dh < 128:
            for h in range(hc):
                zrows = (slice(dh, 128) if h == 0 else slice(0, h * dh))
                nc.vector.memset(q2tz[h][zrows, :].bitcast(F32), 0.0)
        k2t = singles.tile([128, T], MDT, tag="k2t")
        out2t = singles.tile([128, T], MDT, tag="out2t")
        # vnat[:, u, kt, 0:64] = V rows (k on partitions); col 64 = ones
        vnat = singles.tile([128, nb * hc, KT, dh + 1], MDT, tag="vnat")
        onesc = singles.tile([128, nb * hc, KT, 1], F32)
        nc.vector.memset(onesc[:], 1.0)
        nc.vector.tensor_copy(vnat[:, :, :, dh:dh + 1], onesc[:])

        # ---------- emission helpers ----------
        def p12_chunk(n):
            """Load xT chunk n (512 tokens), project to q/k/v, transpose V."""
            if dma_f32r:
                xt_n = xtpool.tile([128, ndc, 512], MDT, tag="xt",
                                   name=f"xt{n}")
                nc.sync.dma_start(out=xt_n[:],
                                  in_=xt_view[:, :, n * 512:(n + 1) * 512])
            else:
                xr = xtpool.tile([128, ndc, 512], F32, tag="xr", name=f"xr{n}")
                nc.sync.dma_start(out=xr[:],
                                  in_=xt_view[:, :, n * 512:(n + 1) * 512])
                xt_n = xtpool.tile([128, ndc, 512], MDT, tag="xt",
                                   name=f"xt{n}")
                nc.vector.tensor_copy(xt_n[:], xr[:])
            for p in range(3):
                ps = pspool.tile([128, 512], F32, tag="ps", name=f"pj{n}_{p}")
                for dc in range(ndc):
                    nc.tensor.matmul(ps[:], w_sb[p][:, dc, :], xt_n[:, dc, :],
                                     start=(dc == 0), stop=(dc == ndc - 1))
                if p == 0:
                    ncol = slice(n * 512, (n + 1) * 512)
                    for h in range(hc):
                        hp_ = slice(h * dh, (h + 1) * dh)
                        if with_qkv_bias:
                            nc.vector.tensor_scalar_add(
                                q2tz[h][hp_, ncol], ps[hp_, :], b_sb[0][hp_, :])
                        else:
                            nc.vector.tensor_copy(q2tz[h][hp_, ncol],
                                                  ps[hp_, :])
                elif p == 1:
                    if with_qkv_bias:
                        nc.vector.tensor_scalar_add(
                            k2t[:, n * 512:(n + 1) * 512], ps[:], b_sb[1][:])
                    else:
                        nc.vector.tensor_copy(k2t[:, n * 512:(n + 1) * 512],
                                              ps[:])
                else:
                    v2_n = v2pool.tile([128, 512], F32, tag="v2")
                    if with_qkv_bias:
                        nc.vector.tensor_scalar_add(v2_n[:], ps[:], b_sb[2][:])
                    else:
                        nc.vector.tensor_copy(v2_n[:], ps[:])
                    b = (n * 512) // S
                    kt0 = (n * 512 % S) // 128
                    pv = pspool.tile([128, 4, 128], F32, tag="ps",
                                     name=f"pv{n}")
                    for sub in range(4):
                        nc.tensor.transpose(
                            pv[:, sub, :], v2_n[:, sub * 128:(sub + 1) * 128],
                            ident[:])
                    for h in range(hc):
                        nc.vector.tensor_copy(
                            vnat[:, b * hc + h, kt0:kt0 + 4, 0:dh],
                            pv[:, :, h * dh:(h + 1) * dh])

        def p4_tile(i):
            """Output-projection tile i (i indexes (m, j) pairs)."""
            m, j = divmod(i, D // NJ)
            py = pspool.tile([128, NJ], F32, tag="ps", name=f"py{i}")
            nc.tensor.matmul(py[:], out2t[:, m * 128:(m + 1) * 128],
                             wo_sb[:, j * NJ:(j + 1) * NJ],
                             start=True, stop=True)
            yt = ysb.tile([128, NJ], F32, tag="yt")
            if with_o_bias:
                nc.vector.tensor_add(yt[:], py[:], bo_sb[:, j * NJ:(j + 1) * NJ])
            else:
                nc.vector.tensor_copy(yt[:], py[:])
            nc.sync.dma_start(out=y_d[m * 128:(m + 1) * 128,
                                      j * NJ:(j + 1) * NJ], in_=yt[:])

        def finish_rec(u, po_sb, qc, rcs):
            """Deferred softmax recip (slow single-partition DVE op; issue
            well before the broadcast matmul that consumes it)."""
            rc = rcpool.tile([1, 512], MDT, tag="rc", bufs=6,
                             name=f"rc{u}_{qc}")
            with nc.allow_low_precision(reason="softmax denom"):
                nc.vector.reciprocal(rc[:], po_sb[dh:dh + 1, qc, :])
            rcs[qc] = rc

        def finish_mul(u, po_sb, qc, rcs):
            """Broadcast 1/denom across dh partitions and normalize."""
            b, h = divmod(u, hc)
            hp = slice(h * dh, (h + 1) * dh)
            qcol = slice(b * S + qc * 512, b * S + (qc + 1) * 512)
            rcp = pspool.tile([dh, 512], F32, tag="ps", name=f"rcp{u}_{qc}")
            nc.tensor.matmul(rcp[:], ones1[:], rcs[qc][:], start=True,
                             stop=True)
            nc.vector.tensor_mul(out2t[hp, qcol], po_sb[0:dh, qc, :], rcp[:])

        # ---------- interleaved emission ----------
        from collections import deque
        units = list(range(nb * hc))
        last_u = units[-1]
        npj = D // NJ  # p4 tiles per m-tile

        def p4_tiles_for(b, qcset=None):
            """p4 tile indices for batch b (optionally only given q-chunks)."""
            out = []
            for m in range(b * S // 128, (b + 1) * S // 128):
                qc = (m * 128 % S) // 512
                if qcset is None or qc in qcset:
                    out.extend(m * npj + j for j in range(npj))
            return out

        # initial projections: just enough chunks to start attention
        ninit = min(2, cpb)
        for n in range(ninit):
            p12_chunk(n)
        rest_chunks = deque(range(ninit, nb * cpb))

        # fillers per unit: remaining projection chunks go to the earliest
        # units; p4 tiles of batch b ride with units of later batches.
        fillers = {u: deque() for u in units}
        b0_units = [u for u in units if u // hc == 0]
        for idx, n in enumerate(rest_chunks):
            fillers[b0_units[idx * len(b0_units) // len(rest_chunks)]
                    ].append(("chunk", n))
        for b in range(nb - 1):
            hosts = [u for u in units if u // hc > b]
            tiles = p4_tiles_for(b)
            for idx, i in enumerate(tiles):
                fillers[hosts[idx * len(hosts) // len(tiles)]].append(("p4", i))

        prev_finish = None  # (u, po_sb, [qcs]) awaiting normalization
        for u in units:
            b, h = divmod(u, hc)
            hp = slice(h * dh, (h + 1) * dh)
            # the last unit runs one q-chunk at a time so its batch's output
            # projection can ride inside the attention stream sooner
            gsz = 1 if u == last_u else 2
            halves = [list(range(q0, min(q0 + gsz, QC)))
                      for q0 in range(0, QC, gsz)]
            nsteps = len(halves) * KT
            work = deque()
            if prev_finish is not None:
                fu, fpo, fqcs, frcs = prev_finish
                work.extend(("mul", fu, fpo, qc, frcs) for qc in fqcs)
            work.extend(fillers[u])
            po_sb = posb.tile([dh + 1, QC, 512], F32, tag="posb",
                              name=f"posb{u}")
            urcs = {}

            step = 0
            for hi_, qcs in enumerate(halves):
                g = len(qcs)
                po = [psO.tile([dh + 1, 512], F32, tag="po",
                               name=f"po{u}_{qc}") for qc in qcs]

                def issue_po(kt, exs):
                    for i in range(g):
                        nc.tensor.matmul(po[i][:], vnat[:, u, kt, :], exs[i],
                                         start=(kt == 0), stop=(kt == KT - 1))

                prev_exs = prev2_exs = None
                for kt in range(KT):
                    kcol = slice(b * S + kt * 128, b * S + (kt + 1) * 128)
                    exs = []
                    for qc in qcs:
                        qcol = slice(b * S + qc * 512, b * S + (qc + 1) * 512)
                        sc1 = pspool.tile([128, 512], F32, tag="sc",
                                          name=f"sc{u}_{qc}_{kt}", bufs=4)
                        nc.tensor.matmul(sc1[:], k2t[:, kcol],
                                         q2tz[h][:, qcol],
                                         start=True, stop=True)
                        ex1 = epool.tile([128, 512], MDT, tag="ex",
                                         name=f"ex{u}_{qc}_{kt}", bufs=6)
                        nc.scalar.activation(ex1[:], sc1[:], AF.Exp,
                                             scale=scale)
                        exs.append(ex1[:])
                    if kt > 1:
                        issue_po(kt - 2, prev2_exs)
                    prev2_exs = prev_exs
                    prev_exs = exs

                    # drain deferred work (finishes first, then fillers),
                    # pacing so the queue empties by the last step
                    steps_left = nsteps - step
                    npop = (len(work) + steps_left - 1) // steps_left
                    for _ in range(min(npop, len(work))):
                        item = work.popleft()
                        if item[0] == "mul":
                            finish_mul(item[1], item[2], item[3], item[4])
                        elif item[0] == "chunk":
                            p12_chunk(item[1])
                        else:
                            p4_tile(item[1])
                    step += 1
                issue_po(KT - 2, prev2_exs)
                issue_po(KT - 1, prev_exs)
                # reciprocal straight from PSUM (slow single-lane DVE op --
                # start it as early as possible), then drain po -> SBUF
                for i, qc in enumerate(qcs):
                    rc = rcpool.tile([1, 512], MDT, tag="rc", bufs=6,
                                     name=f"rc{u}_{qc}")
                    with nc.allow_low_precision(reason="softmax denom"):
                        nc.vector.reciprocal(rc[:], po[i][dh:dh + 1, :])
                    urcs[qc] = rc
                for i, qc in enumerate(qcs):
                    nc.vector.tensor_copy(po_sb[:, qc, :], po[i][:])
                if u == last_u:
                    # last unit: queue normalization at each half boundary
                    # and this batch's output projection right behind it
                    work.extend(("mul", u, po_sb, qc, urcs) for qc in qcs)
                    work.extend(("p4", i) for i in p4_tiles_for(b, set(qcs)))
            if u == last_u:
                # drain any remaining queued work
                while work:
                    item = work.popleft()
                    if item[0] == "mul":
                        finish_mul(item[1], item[2], item[3], item[4])
                    elif item[0] == "chunk":
                        p12_chunk(item[1])
                    else:
                        p4_tile(item[1])
            else:
                prev_finish = (u, po_sb, list(range(QC)), urcs)

    nc.compile()
    return nc


_PROGRAM_CACHE = {}


def _get_program(key):
    if key not in _PROGRAM_CACHE:
        with_qkv_bias, with_o_bias = key
        _PROGRAM_CACHE[key] = build_program(
            with_qkv_bias=with_qkv_bias, with_o_bias=with_o_bias)
    return _PROGRAM_CACHE[key]


def _round_tf32(a):
    """Round fp32 to tf32 (10-bit mantissa), round-to-nearest-even."""
    u = a.view(np.uint32)
    r = (u + 0xFFF + ((u >> 13) & 1)) & np.uint32(0xFFFFE000)
    return r.view(np.float32)


def make_in_maps(x, wq, bq, wk, bk, wv, bv, wo, bo, with_qkv_bias, with_o_bias,
                 n_cores=N_CORES, hc=HEADS_PER_CORE, dh=DEPTH):
    d2 = hc * dh
    xt = _round_tf32(np.ascontiguousarray(x.T))
    in_maps = []
    for c in range(n_cores):
        cs = slice(c * d2, (c + 1) * d2)
        m = {"xt": xt,
             "wq": np.ascontiguousarray(wq[:, cs]),
             "wk": np.ascontiguousarray(wk[:, cs]),
             "wv": np.ascontiguousarray(wv[:, cs]),
             "wo": np.ascontiguousarray(wo[cs, :])}
        if with_qkv_bias:
            m["bq"] = np.ascontiguousarray(bq[cs].reshape(d2, 1))
            m["bk"] = np.ascontiguousarray(bk[cs].reshape(d2, 1))
            m["bv"] = np.ascontiguousarray(bv[cs].reshape(d2, 1))
        if with_o_bias:
            m["bo"] = (bo.reshape(1, -1).astype(np.float32) if c == 0
                       else np.zeros((1, bo.shape[-1]), np.float32))
        in_maps.append(m)
    return in_maps


def kernel(inputs, wq, bq, wk, bk, wv, bv, wo, bo):
    x = np.ascontiguousarray(np.asarray(inputs, np.float32)
                             .reshape(B_FULL * S_FULL, D_MODEL))
    wq, wk, wv, wo = (np.asarray(a, np.float32) for a in (wq, wk, wv, wo))
    bq, bk, bv, bo = (np.asarray(a, np.float32) for a in (bq, bk, bv, bo))

    with_qkv_bias = bool(np.any(bq) or np.any(bk) or np.any(bv))
    with_o_bias = bool(np.any(bo))
    nc = _get_program((with_qkv_bias, with_o_bias))

    in_maps = make_in_maps(x, wq, bq, wk, bk, wv, bv, wo, bo,
                           with_qkv_bias, with_o_bias)
    res = run_bass_kernel_spmd(nc, in_maps, list(range(N_CORES))).results
    y = np.zeros((B_FULL * S_FULL, D_MODEL), np.float64)
    for c in range(N_CORES):
        y += res[c]["y"]
    return y.astype(np.float32).reshape(B_FULL, S_FULL, D_MODEL)


# revision 36
# speedup vs baseline: 1.0069x; 1.0069x over previous
"""Multi-head self-attention (B=2, S=2048, D=1024, H=16) on 8 TRN2 NeuronCores.

Sharding: head-parallel — 2 heads per core. Each core computes Q/K/V
projections for its 2 heads over all B*S tokens, full (non-causal)
softmax attention for its 4 (batch, head) units, and a partial output
projection y_c = sum_h out_h @ wo[h]. Host sums the 8 partial outputs.
The host also pre-transposes x to xT (pure layout prep) so the device
reads the contraction dim on partitions directly.

Device dataflow (head-dim on partitions):
  q2t/k2t/v2t [128=2*64, T] = w[:,2heads]^T @ xT      (PSUM accum over D)
  v2t --PE transpose--> vnat [k, d] (+ ones column -> denominator row)
  scoresT[k, q] = K Q^T  (contract d=64), exp on ACT (scale=1/8 folded in)
  poT[d+1, q] += vnat^T @ exp  accumulated over k tiles  (PSUM)
  out2t[:, q] = poT[0:64] * (1/poT[64]) broadcast (K=1 matmul + DVE mul)
  y[s, n] = out2t[:, s-tile]^T @ wo2   (contract 128 = 2 heads * 64)

Matmuls run as float32r (tfloat32 datapath, 1 cycle/row); every operand
tile is produced with an fp32r-rounding instruction as the BIR verifier
requires. PSUM accumulation stays fp32.

Emission is phase-interleaved so the PE always has independent filler
work while the ACT engine grinds through the exps: projections for
batch 1 ride along with batch 0's attention, and batch 0's output
projection rides along with batch 1's attention.
"""

import numpy as np
from contextlib import ExitStack

import concourse.bass as bass
import concourse.tile as tile
from concourse import bacc, mybir
from concourse.bass_utils import run_bass_kernel_spmd
from concourse.masks import make_identity

F32 = mybir.dt.float32
F32R = mybir.dt.float32r
AF = mybir.ActivationFunctionType

N_CORES = 8
D_MODEL = 1024
NUM_HEADS = 16
DEPTH = 64
HEADS_PER_CORE = NUM_HEADS // N_CORES  # 2
B_FULL = 2
S_FULL = 2048


def build_program(T=4096, D=1024, S=2048, dh=64, hc=2, with_qkv_bias=False,
                  with_o_bias=False, use_f32r=True, dma_f32r=True):
    """Build the SPMD Bass program for one core (hc heads).

    T: total tokens (B*S); D: model dim; S: seq len per batch; dh: head depth;
    hc: heads per core. Requires hc*dh == 128, D % 128 == 0, S % 512 == 0,
    T % S == 0. dma_f32r: DMA x directly into fp32r tiles (if the verifier
    allows DMA producers); else DMA to fp32 and round via DVE copy.
    """
    d2 = hc * dh
    assert d2 == 128 and D % 128 == 0 and S % 512 == 0 and T % S == 0
    nb = T // S            # batches
    ndc = D // 128         # D chunks (contraction tiles)
    cpb = S // 512         # 512-token chunks per batch
    KT = S // 128          # k tiles per (b,h) unit
    QC = S // 512          # 512-wide q chunks per batch
    NJ = min(512, D)
    scale = 1.0 / float(np.sqrt(dh))
    MDT = F32R if use_f32r else F32

    nc = bacc.Bacc("TRN2", target_bir_lowering=False, debug=False,
                   num_devices=N_CORES)

    xt_d = nc.dram_tensor("xt", [D, T], F32R if dma_f32r else F32,
                          kind="ExternalInput").ap()
    wq_d = nc.dram_tensor("wq", [D, d2], F32, kind="ExternalInput").ap()
    wk_d = nc.dram_tensor("wk", [D, d2], F32, kind="ExternalInput").ap()
    wv_d = nc.dram_tensor("wv", [D, d2], F32, kind="ExternalInput").ap()
    wo_d = nc.dram_tensor("wo", [d2, D], F32, kind="ExternalInput").ap()
    if with_qkv_bias:
        bq_d = nc.dram_tensor("bq", [d2, 1], F32, kind="ExternalInput").ap()
        bk_d = nc.dram_tensor("bk", [d2, 1], F32, kind="ExternalInput").ap()
        bv_d = nc.dram_tensor("bv", [d2, 1], F32, kind="ExternalInput").ap()
    if with_o_bias:
        bo_d = nc.dram_tensor("bo", [1, D], F32, kind="ExternalInput").ap()
    y_d = nc.dram_tensor("y", [T, D], F32, kind="ExternalOutput").ap()

    xt_view = xt_d.rearrange("(dc p) t -> p dc t", p=128)

    with tile.TileContext(nc) as tc, ExitStack() as ctx:
        singles = ctx.enter_context(tc.tile_pool(name="singles", bufs=1))
        xtpool = ctx.enter_context(tc.tile_pool(name="xtpool", bufs=3))
        v2pool = ctx.enter_context(tc.tile_pool(name="v2pool", bufs=2))
        epool = ctx.enter_context(tc.tile_pool(name="epool", bufs=4))
        ysb = ctx.enter_context(tc.tile_pool(name="ysb", bufs=3))
        rcpool = ctx.enter_context(tc.tile_pool(name="rcpool", bufs=2))
        # PSUM budget (8 banks): sc 4x[128,512]=4, ps 2x[128,512]=2,
        # po 2x[65,512]=2
        pspool = ctx.enter_context(tc.tile_pool(name="ps", bufs=2, space="PSUM"))
        psO = ctx.enter_context(tc.tile_pool(name="psO", bufs=2, space="PSUM"))
        posb = ctx.enter_context(tc.tile_pool(name="posb", bufs=2))

        ident = singles.tile([128, 128], F32)
        make_identity(nc, ident[:])
        ones1f = singles.tile([1, dh], F32)
        nc.vector.memset(ones1f[:], 1.0)
        ones1 = singles.tile([1, dh], MDT)
        nc.vector.tensor_copy(ones1[:], ones1f[:])

        # weights: load fp32, round on-chip to the matmul dtype
        w_sb = []
        with tc.tile_pool(name="wraw", bufs=2) as wraw:
            for name, wd in (("wqs", wq_d), ("wks", wk_d), ("wvs", wv_d)):
                raw = wraw.tile([128, ndc, d2], F32, tag="wr", name=f"raw_{name}")
                nc.sync.dma_start(out=raw[:],
                                  in_=wd.rearrange("(dc p) m -> p dc m", p=128))
                t = singles.tile([128, ndc, d2], MDT, tag=name, name=name)
                nc.vector.tensor_copy(t[:], raw[:])
                w_sb.append(t)
            raw = wraw.tile([d2, D], F32, tag="wr", name="raw_wo")
            nc.sync.dma_start(out=raw[:], in_=wo_d)
            wo_sb = singles.tile([d2, D], MDT)
            nc.vector.tensor_copy(wo_sb[:], raw[:])

        b_sb = [None, None, None]
        if with_qkv_bias:
            for i, bd in enumerate((bq_d, bk_d, bv_d)):
                t = singles.tile([d2, 1], F32, tag=f"b{i}", name=f"b{i}")
                nc.sync.dma_start(out=t[:], in_=bd)
                b_sb[i] = t
        bo_sb = None
        if with_o_bias:
            bo_sb = singles.tile([128, D], F32)
            nc.gpsimd.dma_start(out=bo_sb[:], in_=bo_d.partition_broadcast(128))

        # Q stored zero-padded per head: q2tz[h] has head h's Q^T on its own
        # 64 partitions and ZEROS on the other 64. The score matmul then runs
        # with the full [128,128] two-head K tile as stationary (K=128
        # contraction) — fp32r at K=64 is half-rate, K=128 is full-rate —
        # and the zero rows cancel the other head's contribution.
        q2tz = [singles.tile([128, T], MDT, tag=f"q2tz{h}", name=f"q2tz{h}")
                for h in range(hc)]
        if dh < 128:
            for h in range(hc):
                zrows = (slice(dh, 128) if h == 0 else slice(0, h * dh))
                nc.vector.memset(q2tz[h][zrows, :].bitcast(F32), 0.0)
        k2t = singles.tile([128, T], MDT, tag="k2t")
        out2t = singles.tile([128, T], MDT, tag="out2t")
        # vnat[:, u, kt, 0:64] = V rows (k on partitions); col 64 = ones
        vnat = singles.tile([128, nb * hc, KT, dh + 1], MDT, tag="vnat")
        onesc = singles.tile([128, nb * hc, KT, 1], F32)
        nc.vector.memset(onesc[:], 1.0)
        nc.vector.tensor_copy(vnat[:, :, :, dh:dh + 1], onesc[:])

        # ---------- emission helpers ----------
        def p12_chunk(n):
            """Load xT chunk n (512 tokens), project to q/k/v, transpose V."""
            if dma_f32r:
                xt_n = xtpool.tile([128, ndc, 512], MDT, tag="xt",
                                   name=f"xt{n}")
                for dc in range(ndc):
                    nc.sync.dma_start(
                        out=xt_n[:, dc, :],
                        in_=xt_view[:, dc, n * 512:(n + 1) * 512])
            else:
                xr = xtpool.tile([128, ndc, 512], F32, tag="xr", name=f"xr{n}")
                nc.sync.dma_start(out=xr[:],
                                  in_=xt_view[:, :, n * 512:(n + 1) * 512])
                xt_n = xtpool.tile([128, ndc, 512], MDT, tag="xt",
                                   name=f"xt{n}")
                nc.vector.tensor_copy(xt_n[:], xr[:])
            for p in range(3):
                ps = pspool.tile([128, 512], F32, tag="ps", name=f"pj{n}_{p}")
                for dc in range(ndc):
                    nc.tensor.matmul(ps[:], w_sb[p][:, dc, :], xt_n[:, dc, :],
                                     start=(dc == 0), stop=(dc == ndc - 1))
                if p == 0:
                    ncol = slice(n * 512, (n + 1) * 512)
                    for h in range(hc):
                        hp_ = slice(h * dh, (h + 1) * dh)
                        if with_qkv_bias:
                            nc.vector.tensor_scalar_add(
                                q2tz[h][hp_, ncol], ps[hp_, :], b_sb[0][hp_, :])
                        else:
                            nc.vector.tensor_copy(q2tz[h][hp_, ncol],
                                                  ps[hp_, :])
                elif p == 1:
                    if with_qkv_bias:
                        nc.vector.tensor_scalar_add(
                            k2t[:, n * 512:(n + 1) * 512], ps[:], b_sb[1][:])
                    else:
                        nc.vector.tensor_copy(k2t[:, n * 512:(n + 1) * 512],
                                              ps[:])
                else:
                    v2_n = v2pool.tile([128, 512], F32, tag="v2")
                    if with_qkv_bias:
                        nc.vector.tensor_scalar_add(v2_n[:], ps[:], b_sb[2][:])
                    else:
                        nc.vector.tensor_copy(v2_n[:], ps[:])
                    b = (n * 512) // S
                    kt0 = (n * 512 % S) // 128
                    pv = pspool.tile([128, 4, 128], F32, tag="ps",
                                     name=f"pv{n}")
                    for sub in range(4):
                        nc.tensor.transpose(
                            pv[:, sub, :], v2_n[:, sub * 128:(sub + 1) * 128],
                            ident[:])
                    for h in range(hc):
                        nc.vector.tensor_copy(
                            vnat[:, b * hc + h, kt0:kt0 + 4, 0:dh],
                            pv[:, :, h * dh:(h + 1) * dh])

        def p4_tile(i):
            """Output-projection tile i (i indexes (m, j) pairs)."""
            m, j = divmod(i, D // NJ)
            py = pspool.tile([128, NJ], F32, tag="ps", name=f"py{i}")
            nc.tensor.matmul(py[:], out2t[:, m * 128:(m + 1) * 128],
                             wo_sb[:, j * NJ:(j + 1) * NJ],
                             start=True, stop=True)
            yt = ysb.tile([128, NJ], F32, tag="yt")
            if with_o_bias:
                nc.vector.tensor_add(yt[:], py[:], bo_sb[:, j * NJ:(j + 1) * NJ])
            else:
                nc.vector.tensor_copy(yt[:], py[:])
            nc.sync.dma_start(out=y_d[m * 128:(m + 1) * 128,
                                      j * NJ:(j + 1) * NJ], in_=yt[:])

        def finish_rec(u, po_sb, qc, rcs):
            """Deferred softmax recip (slow single-partition DVE op; issue
            well before the broadcast matmul that consumes it)."""
            rc = rcpool.tile([1, 512], MDT, tag="rc", bufs=6,
                             name=f"rc{u}_{qc}")
            with nc.allow_low_precision(reason="softmax denom"):
                nc.vector.reciprocal(rc[:], po_sb[dh:dh + 1, qc, :])
            rcs[qc] = rc

        def finish_mul(u, po_sb, qc, rcs):
            """Broadcast 1/denom across dh partitions and normalize."""
            b, h = divmod(u, hc)
            hp = slice(h * dh, (h + 1) * dh)
            qcol = slice(b * S + qc * 512, b * S + (qc + 1) * 512)
            rcp = pspool.tile([dh, 512], F32, tag="ps", name=f"rcp{u}_{qc}")
            nc.tensor.matmul(rcp[:], ones1[:], rcs[qc][:], start=True,
                             stop=True)
            nc.vector.tensor_mul(out2t[hp, qcol], po_sb[0:dh, qc, :], rcp[:])

        # ---------- interleaved emission ----------
        from collections import deque
        units = list(range(nb * hc))
        last_u = units[-1]
        npj = D // NJ  # p4 tiles per m-tile

        def p4_tiles_for(b, qcset=None):
            """p4 tile indices for batch b (optionally only given q-chunks)."""
            out = []
            for m in range(b * S // 128, (b + 1) * S // 128):
                qc = (m * 128 % S) // 512
                if qcset is None or qc in qcset:
                    out.extend(m * npj + j for j in range(npj))
            return out

        # initial projections: just enough chunks to start attention
        ninit = min(2, cpb)
        for n in range(ninit):
            p12_chunk(n)
        rest_chunks = deque(range(ninit, nb * cpb))

        # fillers per unit: remaining projection chunks go to the earliest
        # units; p4 tiles of batch b ride with units of later batches.
        fillers = {u: deque() for u in units}
        b0_units = [u for u in units if u // hc == 0]
        for idx, n in enumerate(rest_chunks):
            fillers[b0_units[idx * len(b0_units) // len(rest_chunks)]
                    ].append(("chunk", n))
        for b in range(nb - 1):
            hosts = [u for u in units if u // hc > b]
            tiles = p4_tiles_for(b)
            for idx, i in enumerate(tiles):
                fillers[hosts[idx * len(hosts) // len(tiles)]].append(("p4", i))

        prev_finish = None  # (u, po_sb, [qcs]) awaiting normalization
        for u in units:
            b, h = divmod(u, hc)
            hp = slice(h * dh, (h + 1) * dh)
            # the last unit runs one q-chunk at a time so its batch's output
            # projection can ride inside the attention stream sooner
            gsz = 1 if u == last_u else 2
            halves = [list(range(q0, min(q0 + gsz, QC)))
                      for q0 in range(0, QC, gsz)]
            nsteps = len(halves) * KT
            work = deque()
            if prev_finish is not None:
                fu, fpo, fqcs, frcs = prev_finish
                work.extend(("mul", fu, fpo, qc, frcs) for qc in fqcs)
            work.extend(fillers[u])
            po_sb = posb.tile([dh + 1, QC, 512], F32, tag="posb",
                              name=f"posb{u}")
            urcs = {}

            step = 0
            for hi_, qcs in enumerate(halves):
                g = len(qcs)
                po = [psO.tile([dh + 1, 512], F32, tag="po",
                               name=f"po{u}_{qc}") for qc in qcs]

                def issue_po(kt, exs):
                    for i in range(g):
                        nc.tensor.matmul(po[i][:], vnat[:, u, kt, :], exs[i],
                                         start=(kt == 0), stop=(kt == KT - 1))

                prev_exs = prev2_exs = None
                for kt in range(KT):
                    kcol = slice(b * S + kt * 128, b * S + (kt + 1) * 128)
                    exs = []
                    for qc in qcs:
                        qcol = slice(b * S + qc * 512, b * S + (qc + 1) * 512)
                        sc1 = pspool.tile([128, 512], F32, tag="sc",
                                          name=f"sc{u}_{qc}_{kt}", bufs=4)
                        nc.tensor.matmul(sc1[:], k2t[:, kcol],
                                         q2tz[h][:, qcol],
                                         start=True, stop=True)
                        ex1 = epool.tile([128, 512], MDT, tag="ex",
                                         name=f"ex{u}_{qc}_{kt}", bufs=6)
                        nc.scalar.activation(ex1[:], sc1[:], AF.Exp,
                                             scale=scale)
                        exs.append(ex1[:])
                    if kt > 1:
                        issue_po(kt - 2, prev2_exs)
                    prev2_exs = prev_exs
                    prev_exs = exs

                    # drain deferred work (finishes first, then fillers),
                    # pacing so the queue empties by the last step
                    steps_left = nsteps - step
                    npop = (len(work) + steps_left - 1) // steps_left
                    for _ in range(min(npop, len(work))):
                        item = work.popleft()
                        if item[0] == "mul":
                            finish_mul(item[1], item[2], item[3], item[4])
                        elif item[0] == "chunk":
                            p12_chunk(item[1])
                        else:
                            p4_tile(item[1])
                    step += 1
                issue_po(KT - 2, prev2_exs)
                issue_po(KT - 1, prev_exs)
                # drain po -> SBUF first (frees the PSUM banks for the next
                # half), then start the slow single-lane reciprocal early
                for i, qc in enumerate(qcs):
                    nc.vector.tensor_copy(po_sb[:, qc, :], po[i][:])
                for i, qc in enumerate(qcs):
                    rc = rcpool.tile([1, 512], MDT, tag="rc", bufs=6,
                                     name=f"rc{u}_{qc}")
                    with nc.allow_low_precision(reason="softmax denom"):
                        nc.vector.reciprocal(rc[:], po_sb[dh:dh + 1, qc, :])
                    urcs[qc] = rc
                if u == last_u:
                    # last unit: queue normalization at each half boundary
                    # and this batch's output projection right behind it
                    work.extend(("mul", u, po_sb, qc, urcs) for qc in qcs)
                    work.extend(("p4", i) for i in p4_tiles_for(b, set(qcs)))
            if u == last_u:
                # drain any remaining queued work
                while work:
                    item = work.popleft()
                    if item[0] == "mul":
                        finish_mul(item[1], item[2], item[3], item[4])
                    elif item[0] == "chunk":
                        p12_chunk(item[1])
                    else:
                        p4_tile(item[1])
            else:
                prev_finish = (u, po_sb, list(range(QC)), urcs)

    nc.compile()
    return nc


_PROGRAM_CACHE = {}


def _get_program(key):
    if key not in _PROGRAM_CACHE:
        with_qkv_bias, with_o_bias = key
        _PROGRAM_CACHE[key] = build_program(
            with_qkv_bias=with_qkv_bias, with_o_bias=with_o_bias)
    return _PROGRAM_CACHE[key]


def _round_tf32(a):
    """Round fp32 to tf32 (10-bit mantissa), round-to-nearest-even."""
    u = a.view(np.uint32)
    r = (u + 0xFFF + ((u >> 13) & 1)) & np.uint32(0xFFFFE000)
    return r.view(np.float32)


def make_in_maps(x, wq, bq, wk, bk, wv, bv, wo, bo, with_qkv_bias, with_o_bias,
                 n_cores=N_CORES, hc=HEADS_PER_CORE, dh=DEPTH):
    d2 = hc * dh
    xt = _round_tf32(np.ascontiguousarray(x.T))
    in_maps = []
    for c in range(n_cores):
        cs = slice(c * d2, (c + 1) * d2)
        m = {"xt": xt,
             "wq": np.ascontiguousarray(wq[:, cs]),
             "wk": np.ascontiguousarray(wk[:, cs]),
             "wv": np.ascontiguousarray(wv[:, cs]),
             "wo": np.ascontiguousarray(wo[cs, :])}
        if with_qkv_bias:
            m["bq"] = np.ascontiguousarray(bq[cs].reshape(d2, 1))
            m["bk"] = np.ascontiguousarray(bk[cs].reshape(d2, 1))
            m["bv"] = np.ascontiguousarray(bv[cs].reshape(d2, 1))
        if with_o_bias:
            m["bo"] = (bo.reshape(1, -1).astype(np.float32) if c == 0
                       else np.zeros((1, bo.shape[-1]), np.float32))
        in_maps.append(m)
    return in_maps


def kernel(inputs, wq, bq, wk, bk, wv, bv, wo, bo):
    x = np.ascontiguousarray(np.asarray(inputs, np.float32)
                             .reshape(B_FULL * S_FULL, D_MODEL))
    wq, wk, wv, wo = (np.asarray(a, np.float32) for a in (wq, wk, wv, wo))
    bq, bk, bv, bo = (np.asarray(a, np.float32) for a in (bq, bk, bv, bo))

    with_qkv_bias = bool(np.any(bq) or np.any(bk) or np.any(bv))
    with_o_bias = bool(np.any(bo))
    nc = _get_program((with_qkv_bias, with_o_bias))

    in_maps = make_in_maps(x, wq, bq, wk, bk, wv, bv, wo, bo,
                           with_qkv_bias, with_o_bias)
    res = run_bass_kernel_spmd(nc, in_maps, list(range(N_CORES))).results
    y = np.zeros((B_FULL * S_FULL, D_MODEL), np.float64)
    for c in range(N_CORES):
        y += res[c]["y"]
    return y.astype(np.float32).reshape(B_FULL, S_FULL, D_MODEL)


# revision 38
# speedup vs baseline: 1.0849x; 1.0774x over previous
"""Multi-head self-attention (B=2, S=2048, D=1024, H=16) on 8 TRN2 NeuronCores.

Sharding: head-parallel — 2 heads per core. Each core computes Q/K/V
projections for its 2 heads over all B*S tokens, full (non-causal)
softmax attention for its 4 (batch, head) units, and a partial output
projection y_c = sum_h out_h @ wo[h]. Host sums the 8 partial outputs.
The host also pre-transposes x to xT (pure layout prep) so the device
reads the contraction dim on partitions directly.

Device dataflow (head-dim on partitions):
  q2t/k2t/v2t [128=2*64, T] = w[:,2heads]^T @ xT      (PSUM accum over D)
  v2t --PE transpose--> vnat [k, d] (+ ones column -> denominator row)
  scoresT[k, q] = K Q^T  (contract d=64), exp on ACT (scale=1/8 folded in)
  poT[d+1, q] += vnat^T @ exp  accumulated over k tiles  (PSUM)
  out2t[:, q] = poT[0:64] * (1/poT[64]) broadcast (K=1 matmul + DVE mul)
  y[s, n] = out2t[:, s-tile]^T @ wo2   (contract 128 = 2 heads * 64)

Matmuls run as float32r (tfloat32 datapath, 1 cycle/row); every operand
tile is produced with an fp32r-rounding instruction as the BIR verifier
requires. PSUM accumulation stays fp32.

Emission is phase-interleaved so the PE always has independent filler
work while the ACT engine grinds through the exps: projections for
batch 1 ride along with batch 0's attention, and batch 0's output
projection rides along with batch 1's attention.
"""

import numpy as np
from contextlib import ExitStack

import concourse.bass as bass
import concourse.tile as tile
from concourse import bacc, mybir
from concourse.bass_utils import run_bass_kernel_spmd
from concourse.masks import make_identity

F32 = mybir.dt.float32
F32R = mybir.dt.float32r
AF = mybir.ActivationFunctionType

N_CORES = 8
D_MODEL = 1024
NUM_HEADS = 16
DEPTH = 64
HEADS_PER_CORE = NUM_HEADS // N_CORES  # 2
B_FULL = 2
S_FULL = 2048


def build_program(T=4096, D=1024, S=2048, dh=64, hc=2, with_qkv_bias=False,
                  with_o_bias=False, use_f32r=True, dma_f32r=True):
    """Build the SPMD Bass program for one core (hc heads).

    T: total tokens (B*S); D: model dim; S: seq len per batch; dh: head depth;
    hc: heads per core. Requires hc*dh == 128, D % 128 == 0, S % 512 == 0,
    T % S == 0. dma_f32r: DMA x directly into fp32r tiles (if the verifier
    allows DMA producers); else DMA to fp32 and round via DVE copy.
    """
    d2 = hc * dh
    assert d2 == 128 and D % 128 == 0 and S % 512 == 0 and T % S == 0
    nb = T // S            # batches
    ndc = D // 128         # D chunks (contraction tiles)
    cpb = S // 512         # 512-token chunks per batch
    KT = S // 128          # k tiles per (b,h) unit
    QC = S // 512          # 512-wide q chunks per batch
    NJ = min(512, D)
    scale = 1.0 / float(np.sqrt(dh))
    MDT = F32R if use_f32r else F32

    nc = bacc.Bacc("TRN2", target_bir_lowering=False, debug=False,
                   num_devices=N_CORES)

    xt_d = nc.dram_tensor("xt", [D, T], F32R if dma_f32r else F32,
                          kind="ExternalInput").ap()
    wq_d = nc.dram_tensor("wq", [D, d2], F32, kind="ExternalInput").ap()
    wk_d = nc.dram_tensor("wk", [D, d2], F32, kind="ExternalInput").ap()
    wv_d = nc.dram_tensor("wv", [D, d2], F32, kind="ExternalInput").ap()
    wo_d = nc.dram_tensor("wo", [d2, D], F32, kind="ExternalInput").ap()
    if with_qkv_bias:
        bq_d = nc.dram_tensor("bq", [d2, 1], F32, kind="ExternalInput").ap()
        bk_d = nc.dram_tensor("bk", [d2, 1], F32, kind="ExternalInput").ap()
        bv_d = nc.dram_tensor("bv", [d2, 1], F32, kind="ExternalInput").ap()
    if with_o_bias:
        bo_d = nc.dram_tensor("bo", [1, D], F32, kind="ExternalInput").ap()
    y_d = nc.dram_tensor("y", [T, D], F32, kind="ExternalOutput").ap()

    xt_view = xt_d.rearrange("(dc p) t -> p dc t", p=128)

    with tile.TileContext(nc) as tc, ExitStack() as ctx:
        singles = ctx.enter_context(tc.tile_pool(name="singles", bufs=1))
        xtpool = ctx.enter_context(tc.tile_pool(name="xtpool", bufs=3))
        v2pool = ctx.enter_context(tc.tile_pool(name="v2pool", bufs=2))
        epool = ctx.enter_context(tc.tile_pool(name="epool", bufs=4))
        ysb = ctx.enter_context(tc.tile_pool(name="ysb", bufs=3))
        rcpool = ctx.enter_context(tc.tile_pool(name="rcpool", bufs=2))
        # PSUM budget (8 banks): sc 4x[128,512]=4, ps 2x[128,512]=2,
        # po 2x[65,512]=2
        pspool = ctx.enter_context(tc.tile_pool(name="ps", bufs=2, space="PSUM"))
        psO = ctx.enter_context(tc.tile_pool(name="psO", bufs=2, space="PSUM"))
        posb = ctx.enter_context(tc.tile_pool(name="posb", bufs=2))

        ident = singles.tile([128, 128], F32)
        make_identity(nc, ident[:])
        ones1f = singles.tile([1, dh], F32)
        nc.vector.memset(ones1f[:], 1.0)
        ones1 = singles.tile([1, dh], MDT)
        nc.vector.tensor_copy(ones1[:], ones1f[:])

        # weights: load fp32, round on-chip to the matmul dtype
        w_sb = []
        with tc.tile_pool(name="wraw", bufs=2) as wraw:
            for name, wd in (("wqs", wq_d), ("wks", wk_d), ("wvs", wv_d)):
                raw = wraw.tile([128, ndc, d2], F32, tag="wr", name=f"raw_{name}")
                nc.sync.dma_start(out=raw[:],
                                  in_=wd.rearrange("(dc p) m -> p dc m", p=128))
                t = singles.tile([128, ndc, d2], MDT, tag=name, name=name)
                nc.vector.tensor_copy(t[:], raw[:])
                w_sb.append(t)
            raw = wraw.tile([d2, D], F32, tag="wr", name="raw_wo")
            nc.sync.dma_start(out=raw[:], in_=wo_d)
            wo_sb = singles.tile([d2, D], MDT)
            nc.vector.tensor_copy(wo_sb[:], raw[:])

        b_sb = [None, None, None]
        if with_qkv_bias:
            for i, bd in enumerate((bq_d, bk_d, bv_d)):
                t = singles.tile([d2, 1], F32, tag=f"b{i}", name=f"b{i}")
                nc.sync.dma_start(out=t[:], in_=bd)
                b_sb[i] = t
        bo_sb = None
        if with_o_bias:
            bo_sb = singles.tile([128, D], F32)
            nc.gpsimd.dma_start(out=bo_sb[:], in_=bo_d.partition_broadcast(128))

        # Q stored zero-padded per head: q2tz[h] has head h's Q^T on its own
        # 64 partitions and ZEROS on the other 64. The score matmul then runs
        # with the full [128,128] two-head K tile as stationary (K=128
        # contraction) — fp32r at K=64 is half-rate, K=128 is full-rate —
        # and the zero rows cancel the other head's contribution.
        q2tz = [singles.tile([128, T], MDT, tag=f"q2tz{h}", name=f"q2tz{h}")
                for h in range(hc)]
        if dh < 128:
            for h in range(hc):
                zrows = (slice(dh, 128) if h == 0 else slice(0, h * dh))
                nc.vector.memset(q2tz[h][zrows, :].bitcast(F32), 0.0)
        k2t = singles.tile([128, T], MDT, tag="k2t")
        out2t = singles.tile([128, T], MDT, tag="out2t")
        # vnat[:, u, kt, 0:64] = V rows (k on partitions); col 64 = ones
        vnat = singles.tile([128, nb * hc, KT, dh + 1], MDT, tag="vnat")
        onesc = singles.tile([128, nb * hc, KT, 1], F32)
        nc.vector.memset(onesc[:], 1.0)
        nc.vector.tensor_copy(vnat[:, :, :, dh:dh + 1], onesc[:])

        # ---------- emission helpers ----------
        def p12_chunk(n):
            """Load xT chunk n (512 tokens), project to q/k/v, transpose V."""
            if dma_f32r:
                xt_n = xtpool.tile([128, ndc, 512], MDT, tag="xt",
                                   name=f"xt{n}")
                for dc in range(ndc):
                    nc.sync.dma_start(
                        out=xt_n[:, dc, :],
                        in_=xt_view[:, dc, n * 512:(n + 1) * 512])
            else:
                xr = xtpool.tile([128, ndc, 512], F32, tag="xr", name=f"xr{n}")
                nc.sync.dma_start(out=xr[:],
                                  in_=xt_view[:, :, n * 512:(n + 1) * 512])
                xt_n = xtpool.tile([128, ndc, 512], MDT, tag="xt",
                                   name=f"xt{n}")
                nc.vector.tensor_copy(xt_n[:], xr[:])
            for p in range(3):
                ps = pspool.tile([128, 512], F32, tag="ps", name=f"pj{n}_{p}")
                for dc in range(ndc):
                    nc.tensor.matmul(ps[:], w_sb[p][:, dc, :], xt_n[:, dc, :],
                                     start=(dc == 0), stop=(dc == ndc - 1))
                if p == 0:
                    ncol = slice(n * 512, (n + 1) * 512)
                    for h in range(hc):
                        hp_ = slice(h * dh, (h + 1) * dh)
                        if with_qkv_bias:
                            nc.vector.tensor_scalar_add(
                                q2tz[h][hp_, ncol], ps[hp_, :], b_sb[0][hp_, :])
                        else:
                            nc.vector.tensor_copy(q2tz[h][hp_, ncol],
                                                  ps[hp_, :])
                elif p == 1:
                    if with_qkv_bias:
                        nc.vector.tensor_scalar_add(
                            k2t[:, n * 512:(n + 1) * 512], ps[:], b_sb[1][:])
                    else:
                        nc.vector.tensor_copy(k2t[:, n * 512:(n + 1) * 512],
                                              ps[:])
                else:
                    v2_n = v2pool.tile([128, 512], F32, tag="v2")
                    if with_qkv_bias:
                        nc.vector.tensor_scalar_add(v2_n[:], ps[:], b_sb[2][:])
                    else:
                        nc.vector.tensor_copy(v2_n[:], ps[:])
                    b = (n * 512) // S
                    kt0 = (n * 512 % S) // 128
                    pv = pspool.tile([128, 4, 128], F32, tag="ps",
                                     name=f"pv{n}")
                    for sub in range(4):
                        nc.tensor.transpose(
                            pv[:, sub, :], v2_n[:, sub * 128:(sub + 1) * 128],
                            ident[:])
                    for h in range(hc):
                        nc.vector.tensor_copy(
                            vnat[:, b * hc + h, kt0:kt0 + 4, 0:dh],
                            pv[:, :, h * dh:(h + 1) * dh])

        def p4_tile(i):
            """Output-projection tile i (i indexes (m, j) pairs)."""
            m, j = divmod(i, D // NJ)
            py = pspool.tile([128, NJ], F32, tag="ps", name=f"py{i}")
            nc.tensor.matmul(py[:], out2t[:, m * 128:(m + 1) * 128],
                             wo_sb[:, j * NJ:(j + 1) * NJ],
                             start=True, stop=True)
            yt = ysb.tile([128, NJ], F32, tag="yt")
            if with_o_bias:
                nc.vector.tensor_add(yt[:], py[:], bo_sb[:, j * NJ:(j + 1) * NJ])
            else:
                nc.vector.tensor_copy(yt[:], py[:])
            nc.sync.dma_start(out=y_d[m * 128:(m + 1) * 128,
                                      j * NJ:(j + 1) * NJ], in_=yt[:])

        def finish_mul(u, po_sb, qc):
            """Normalize: broadcast the RAW denominator across dh partitions
            with a K=1 matmul (PE waits only on po_sb, never on a slow DVE
            reciprocal), take the reciprocal on all 64 lanes of the
            broadcast, then scale the numerator."""
            b, h = divmod(u, hc)
            hp = slice(h * dh, (h + 1) * dh)
            qcol = slice(b * S + qc * 512, b * S + (qc + 1) * 512)
            dr = rcpool.tile([1, 512], MDT, tag="dr", bufs=4,
                             name=f"dr{u}_{qc}")
            nc.vector.tensor_copy(dr[:], po_sb[dh:dh + 1, qc, :])
            den = pspool.tile([dh, 512], F32, tag="ps", name=f"den{u}_{qc}")
            nc.tensor.matmul(den[:], ones1[:], dr[:], start=True, stop=True)
            rci = rcpool.tile([dh, 512], F32, tag="rci", bufs=4,
                              name=f"rci{u}_{qc}")
            nc.vector.reciprocal(rci[:], den[:])
            nc.vector.tensor_mul(out2t[hp, qcol], po_sb[0:dh, qc, :], rci[:])

        # ---------- interleaved emission ----------
        from collections import deque
        units = list(range(nb * hc))
        last_u = units[-1]
        npj = D // NJ  # p4 tiles per m-tile

        def p4_tiles_for(b, qcset=None):
            """p4 tile indices for batch b (optionally only given q-chunks)."""
            out = []
            for m in range(b * S // 128, (b + 1) * S // 128):
                qc = (m * 128 % S) // 512
                if qcset is None or qc in qcset:
                    out.extend(m * npj + j for j in range(npj))
            return out

        # initial projections: just enough chunks to start attention
        ninit = min(2, cpb)
        for n in range(ninit):
            p12_chunk(n)
        rest_chunks = deque(range(ninit, nb * cpb))

        # fillers per unit: remaining projection chunks go to the earliest
        # units; p4 tiles of batch b ride with units of later batches.
        fillers = {u: deque() for u in units}
        b0_units = [u for u in units if u // hc == 0]
        for idx, n in enumerate(rest_chunks):
            fillers[b0_units[idx * len(b0_units) // len(rest_chunks)]
                    ].append(("chunk", n))
        for b in range(nb - 1):
            hosts = [u for u in units if u // hc > b]
            tiles = p4_tiles_for(b)
            for idx, i in enumerate(tiles):
                fillers[hosts[idx * len(hosts) // len(tiles)]].append(("p4", i))

        prev_finish = None  # (u, po_sb, [qcs]) awaiting normalization
        for u in units:
            b, h = divmod(u, hc)
            hp = slice(h * dh, (h + 1) * dh)
            # the last unit runs one q-chunk at a time so its batch's output
            # projection can ride inside the attention stream sooner
            gsz = 1 if u == last_u else 2
            halves = [list(range(q0, min(q0 + gsz, QC)))
                      for q0 in range(0, QC, gsz)]
            nsteps = len(halves) * KT
            work = deque()
            if prev_finish is not None:
                fu, fpo, fqcs = prev_finish
                work.extend(("mul", fu, fpo, qc) for qc in fqcs)
            work.extend(fillers[u])
            po_sb = posb.tile([dh + 1, QC, 512], MDT, tag="posb",
                              name=f"posb{u}")

            step = 0
            for hi_, qcs in enumerate(halves):
                g = len(qcs)
                po = [psO.tile([dh + 1, 512], F32, tag="po",
                               name=f"po{u}_{qc}") for qc in qcs]

                def issue_po(kt, exs):
                    for i in range(g):
                        nc.tensor.matmul(po[i][:], vnat[:, u, kt, :], exs[i],
                                         start=(kt == 0), stop=(kt == KT - 1))

                prev_exs = prev2_exs = None
                for kt in range(KT):
                    kcol = slice(b * S + kt * 128, b * S + (kt + 1) * 128)
                    exs = []
                    for qc in qcs:
                        qcol = slice(b * S + qc * 512, b * S + (qc + 1) * 512)
                        sc1 = pspool.tile([128, 512], F32, tag="sc",
                                          name=f"sc{u}_{qc}_{kt}", bufs=4)
                        nc.tensor.matmul(sc1[:], k2t[:, kcol],
                                         q2tz[h][:, qcol],
                                         start=True, stop=True)
                        ex1 = epool.tile([128, 512], MDT, tag="ex",
                                         name=f"ex{u}_{qc}_{kt}", bufs=6)
                        nc.scalar.activation(ex1[:], sc1[:], AF.Exp,
                                             scale=scale)
                        exs.append(ex1[:])
                    if kt > 1:
                        issue_po(kt - 2, prev2_exs)
                    prev2_exs = prev_exs
                    prev_exs = exs

                    # drain deferred work (finishes first, then fillers),
                    # pacing so the queue empties by the last step
                    steps_left = nsteps - step
                    npop = (len(work) + steps_left - 1) // steps_left
                    for _ in range(min(npop, len(work))):
                        item = work.popleft()
                        if item[0] == "mul":
                            finish_mul(item[1], item[2], item[3])
                        elif item[0] == "chunk":
                            p12_chunk(item[1])
                        else:
                            p4_tile(item[1])
                    step += 1
                issue_po(KT - 2, prev2_exs)
                issue_po(KT - 1, prev_exs)
                # drain po -> SBUF (frees the PSUM banks for the next half)
                for i, qc in enumerate(qcs):
                    nc.vector.tensor_copy(po_sb[:, qc, :], po[i][:])
                if u == last_u:
                    # last unit: queue normalization at each half boundary
                    # and this batch's output projection right behind it
                    work.extend(("mul", u, po_sb, qc) for qc in qcs)
                    work.extend(("p4", i) for i in p4_tiles_for(b, set(qcs)))
            if u == last_u:
                # drain any remaining queued work
                while work:
                    item = work.popleft()
                    if item[0] == "mul":
                        finish_mul(item[1], item[2], item[3])
                    elif item[0] == "chunk":
                        p12_chunk(item[1])
                    else:
                        p4_tile(item[1])
            else:
                prev_finish = (u, po_sb, list(range(QC)))

    nc.compile()
    return nc


_PROGRAM_CACHE = {}


def _get_program(key):
    if key not in _PROGRAM_CACHE:
        with_qkv_bias, with_o_bias = key
        _PROGRAM_CACHE[key] = build_program(
            with_qkv_bias=with_qkv_bias, with_o_bias=with_o_bias)
    return _PROGRAM_CACHE[key]


def _round_tf32(a):
    """Round fp32 to tf32 (10-bit mantissa), round-to-nearest-even."""
    u = a.view(np.uint32)
    r = (u + 0xFFF + ((u >> 13) & 1)) & np.uint32(0xFFFFE000)
    return r.view(np.float32)


def make_in_maps(x, wq, bq, wk, bk, wv, bv, wo, bo, with_qkv_bias, with_o_bias,
                 n_cores=N_CORES, hc=HEADS_PER_CORE, dh=DEPTH):
    d2 = hc * dh
    xt = _round_tf32(np.ascontiguousarray(x.T))
    in_maps = []
    for c in range(n_cores):
        cs = slice(c * d2, (c + 1) * d2)
        m = {"xt": xt,
             "wq": np.ascontiguousarray(wq[:, cs]),
             "wk": np.ascontiguousarray(wk[:, cs]),
             "wv": np.ascontiguousarray(wv[:, cs]),
             "wo": np.ascontiguousarray(wo[cs, :])}
        if with_qkv_bias:
            m["bq"] = np.ascontiguousarray(bq[cs].reshape(d2, 1))
            m["bk"] = np.ascontiguousarray(bk[cs].reshape(d2, 1))
            m["bv"] = np.ascontiguousarray(bv[cs].reshape(d2, 1))
        if with_o_bias:
            m["bo"] = (bo.reshape(1, -1).astype(np.float32) if c == 0
                       else np.zeros((1, bo.shape[-1]), np.float32))
        in_maps.append(m)
    return in_maps


def kernel(inputs, wq, bq, wk, bk, wv, bv, wo, bo):
    x = np.ascontiguousarray(np.asarray(inputs, np.float32)
                             .reshape(B_FULL * S_FULL, D_MODEL))
    wq, wk, wv, wo = (np.asarray(a, np.float32) for a in (wq, wk, wv, wo))
    bq, bk, bv, bo = (np.asarray(a, np.float32) for a in (bq, bk, bv, bo))

    with_qkv_bias = bool(np.any(bq) or np.any(bk) or np.any(bv))
    with_o_bias = bool(np.any(bo))
    nc = _get_program((with_qkv_bias, with_o_bias))

    in_maps = make_in_maps(x, wq, bq, wk, bk, wv, bv, wo, bo,
                           with_qkv_bias, with_o_bias)
    res = run_bass_kernel_spmd(nc, in_maps, list(range(N_CORES))).results
    y = np.zeros((B_FULL * S_FULL, D_MODEL), np.float64)
    for c in range(N_CORES):
        y += res[c]["y"]
    return y.astype(np.float32).reshape(B_FULL, S_FULL, D_MODEL)
